# revision 5
# baseline (speedup 1.0000x reference)
"""Trainium2 Bass kernel for NormGatedLinear (RMSNorm + gate linear).

Computes, for x [8192, 7168] f32, norm_weight [7168] f32, gate_weight [384, 7168] f32:
    normed_x = x * rsqrt(mean(x^2, -1) + 1e-6) * norm_weight      [8192, 7168] f32
    logits   = normed_x @ gate_weight.T                            [8192, 384] f32

Sharding: tokens split across 8 NeuronCores (1024 tokens/core); gate weight +
norm weight replicated.  Device layout is hidden-major (x pre-transposed on
host) so the contraction dim sits on SBUF partitions:
  - Sum-of-squares via a Gram matmul diag (PE) instead of DVE reductions.
  - Logits = raw-x @ (W*nw).T in bf16, scaled afterwards by the per-token
    rsqrt factor (exact fp32) on ScalarE.
  - normed_x = x * (nw) * s in fp32 on DVE, in place, streamed back out.
  - norm_weight == 1 fast path: the nw multiply is skipped on device (the
    fold into W' is still applied on host, which is exact for ones).
"""
import sys
sys.path.insert(0, '/opt/trn_rl_repo')

import numpy as np
import ml_dtypes
from contextlib import ExitStack, nullcontext

import concourse.bacc as bacc
import concourse.tile as tile
from concourse import mybir
from concourse.bass_utils import run_bass_kernel_spmd

T_FULL = 8192
H = 7168
NE = 384
NCORES = 8
T_CORE = T_FULL // NCORES      # 1024
C = 256                        # tokens per chunk
NCHUNK = T_CORE // C           # 4
NK = H // 128                  # 56
GRP = 4                        # k-tiles per cast / normalize group
NGRP = NK // GRP
EPS = 1e-6

TRACE = False
LAST_EXEC_NS = None
_NC_CACHE = {}


def _build_nc(repeat: int = 1, apply_nw: bool = False):
    nc = bacc.Bacc("TRN2", target_bir_lowering=False, debug=False, num_devices=1)
    xt = nc.dram_tensor("xt", [H, T_CORE], mybir.dt.float32, kind="ExternalInput").ap()
    wtb = nc.dram_tensor("wtb", [H, NE], mybir.dt.bfloat16, kind="ExternalInput").ap()
    nw_sb_d = nc.dram_tensor("nw_sb", [128, NK], mybir.dt.float32, kind="ExternalInput").ap()
    ident_d = nc.dram_tensor("ident", [128, 128], mybir.dt.float32, kind="ExternalInput").ap()
    ones_d = nc.dram_tensor("ones_r", [1, 128], mybir.dt.float32, kind="ExternalInput").ap()

    normt = nc.dram_tensor("normt", [H, T_CORE], mybir.dt.float32, kind="ExternalOutput").ap()
    logits = nc.dram_tensor("logits", [T_CORE, NE], mybir.dt.float32, kind="ExternalOutput").ap()

    xt3 = xt.rearrange("(k p) t -> p k t", p=128)
    normt3 = normt.rearrange("(k p) t -> p k t", p=128)

    with tile.TileContext(nc) as tc, ExitStack() as ctx:
        cp = ctx.enter_context(tc.tile_pool(name="consts", bufs=1))
        wp = ctx.enter_context(tc.tile_pool(name="w", bufs=1))
        xp = ctx.enter_context(tc.tile_pool(name="xch", bufs=2))
        bp = ctx.enter_context(tc.tile_pool(name="bcast", bufs=3))
        sp = ctx.enter_context(tc.tile_pool(name="small", bufs=3))
        lp = ctx.enter_context(tc.tile_pool(name="lout", bufs=3))
        pl = ctx.enter_context(tc.tile_pool(name="ps_l", bufs=2, space="PSUM"))
        pg = ctx.enter_context(tc.tile_pool(name="ps_g", bufs=1, space="PSUM"))
        pt = ctx.enter_context(tc.tile_pool(name="ps_t", bufs=2, space="PSUM"))
        pb = ctx.enter_context(tc.tile_pool(name="ps_b", bufs=1, space="PSUM"))

        wb = wp.tile([128, NK * NE], mybir.dt.bfloat16)
        nc.sync.dma_start(wb[:].rearrange("p (k e) -> p k e", k=NK),
                          wtb.rearrange("(k p) e -> p k e", p=128))
        nw_sb = cp.tile([128, NK], mybir.dt.float32)
        nc.sync.dma_start(nw_sb[:], nw_sb_d)
        ident = cp.tile([128, 128], mybir.dt.float32)
        nc.sync.dma_start(ident[:], ident_d)
        ones_r = cp.tile([1, 128], mybir.dt.float32)
        nc.sync.dma_start(ones_r[:], ones_d)

        loop_cm = tc.For_i(0, repeat, 1) if repeat > 1 else nullcontext()
        with loop_cm:
          for c in range(NCHUNK):
            t0c = c * C
            xc = xp.tile([128, NK * C], mybir.dt.float32, tag="xc", name=f"xc{c}")
            # split the chunk load so casts can start early
            QK = NK // 4
            for q in range(4):
                nc.sync.dma_start(
                    xc[:, q * QK * C:(q + 1) * QK * C].rearrange("p (k t) -> p k t", k=QK),
                    xt3[:, q * QK:(q + 1) * QK, t0c:t0c + C])

            # ---- bf16 cast in GRP-k groups (ScalarE)
            xbs = []
            for g in range(NGRP):
                xb = bp.tile([128, GRP * C], mybir.dt.bfloat16, tag="xb", name=f"xb{c}_{g}")
                nc.scalar.copy(xb[:], xc[:, g * GRP * C:(g + 1) * GRP * C])
                xbs.append(xb)

            def xbt(k):          # bf16 x tile k: [128, C]
                return xbs[k // GRP][:, (k % GRP) * C:(k % GRP + 1) * C]

            # ---- Gram diag: d[t] = sum_h x[t,h]^2 (two 128-token halves)
            ps_g = [pg.tile([128, C], mybir.dt.float32, tag=f"gram{h}", name=f"gram{c}_{h}")
                    for h in range(2)]
            for k in range(NK):
                for h in range(2):
                    nc.tensor.matmul(ps_g[h][:], xbt(k)[:, h * 128:(h + 1) * 128], xbt(k),
                                     start=(k == 0), stop=(k == NK - 1))
            d2 = sp.tile([128, 2], mybir.dt.float32, tag="d2", name=f"d2{c}")
            scratch = sp.tile([128, 128], mybir.dt.float32, tag="scr", name=f"scr{c}")
            for h in range(2):
                nc.vector.tensor_mul(scratch[:], ps_g[h][:, h * 128:(h + 1) * 128], ident[:])
                nc.vector.reduce_sum(d2[:, h:h + 1], scratch[:], axis=mybir.AxisListType.X)

            # ---- s = 1/sqrt(d/H + eps)
            eps_t = sp.tile([128, 1], mybir.dt.float32, tag="eps", name=f"eps{c}")
            nc.vector.memset(eps_t[:], EPS)
            sq = sp.tile([128, 2], mybir.dt.float32, tag="sq", name=f"sq{c}")
            nc.scalar.activation(sq[:], d2[:], mybir.ActivationFunctionType.Sqrt,
                                 scale=1.0 / H, bias=eps_t[:, 0:1])
            s2 = sp.tile([128, 2], mybir.dt.float32, tag="s2", name=f"s2{c}")
            nc.vector.reciprocal(s2[:], sq[:])

            # ---- transpose s to a row and broadcast to all partitions
            srow = sp.tile([1, C], mybir.dt.float32, tag="srow", name=f"srow{c}")
            for h in range(2):
                ps_t = pt.tile([1, 128], mybir.dt.float32, tag="pst", name=f"pst{c}_{h}")
                nc.tensor.transpose(ps_t[:], s2[:, h:h + 1], ident[:])
                nc.scalar.copy(srow[0:1, h * 128:(h + 1) * 128], ps_t[0:1, :])
            ps_b = pb.tile([128, C], mybir.dt.float32, tag="psb", name=f"psb{c}")
            nc.tensor.matmul(ps_b[:], ones_r[0:1, :], srow[0:1, :], start=True, stop=True)
            # s replicated along free dim so normalize groups use one plain TT
            s_wide = sp.tile([128, GRP * C], mybir.dt.float32, tag="swide", name=f"swide{c}")
            for j in range(GRP):
                nc.scalar.copy(s_wide[:, j * C:(j + 1) * C], ps_b[:])

            # ---- logits: raw-x @ W'(bf16), then scale rows by s (fp32, ScalarE)
            for h in range(2):
                ps_l = pl.tile([128, NE], mybir.dt.float32, tag="psl", name=f"psl{c}_{h}")
                for k in range(NK):
                    nc.tensor.matmul(ps_l[:], xbt(k)[:, h * 128:(h + 1) * 128],
                                     wb[:, k * NE:(k + 1) * NE],
                                     start=(k == 0), stop=(k == NK - 1))
                lsb = lp.tile([128, NE], mybir.dt.float32, tag="lsb", name=f"lsb{c}_{h}")
                nc.scalar.activation(lsb[:], ps_l[:], mybir.ActivationFunctionType.Copy,
                                     scale=s2[:, h:h + 1])
                nc.sync.dma_start(logits[t0c + h * 128: t0c + (h + 1) * 128, :], lsb[:])

            # ---- normed = (x [* nw]) * s, in place, then store chunk in halves
            for g in range(NGRP):
                sl = slice(g * GRP * C, (g + 1) * GRP * C)
                if apply_nw:
                    for j in range(GRP):
                        k = g * GRP + j
                        ssl = slice(k * C, (k + 1) * C)
                        nc.vector.tensor_scalar(out=xc[:, ssl], in0=xc[:, ssl],
                                                scalar1=nw_sb[:, k:k + 1],
                                                scalar2=None, op0=mybir.AluOpType.mult)
                nc.vector.tensor_mul(xc[:, sl], xc[:, sl], s_wide[:])
            HK = NK // 2
            for half in range(2):
                nc.sync.dma_start(
                    normt3[:, half * HK:(half + 1) * HK, t0c:t0c + C],
                    xc[:, half * HK * C:(half + 1) * HK * C].rearrange(
                        "p (k t) -> p k t", k=HK))
    nc.compile()
    return nc


def kernel(x, norm_weight, gate_weight):
    global LAST_EXEC_NS
    x = np.asarray(x)
    norm_weight = np.asarray(norm_weight, dtype=np.float32)
    gate_weight = np.asarray(gate_weight)

    apply_nw = not bool(np.all(norm_weight == 1.0))
    if apply_nw not in _NC_CACHE:
        _NC_CACHE[apply_nw] = _build_nc(apply_nw=apply_nw)
    nc = _NC_CACHE[apply_nw]

    wtb_np = np.ascontiguousarray(
        (gate_weight.astype(np.float32) * norm_weight[None, :]).T
    ).astype(ml_dtypes.bfloat16)
    nw_sb_np = np.ascontiguousarray(norm_weight.reshape(NK, 128).T)
    ident_np = np.eye(128, dtype=np.float32)
    ones_np = np.ones((1, 128), dtype=np.float32)

    in_maps = []
    for i in range(NCORES):
        xt_i = np.ascontiguousarray(x[i * T_CORE:(i + 1) * T_CORE].T)
        in_maps.append({"xt": xt_i, "wtb": wtb_np, "nw_sb": nw_sb_np,
                        "ident": ident_np, "ones_r": ones_np})

    res = run_bass_kernel_spmd(nc, in_maps, core_ids=list(range(NCORES)), trace=TRACE)
    LAST_EXEC_NS = res.exec_time_ns

    normed = np.empty((T_FULL, H), dtype=np.float32)
    logits = np.empty((T_FULL, NE), dtype=np.float32)
    for i in range(NCORES):
        r = res.results[i]
        normed[i * T_CORE:(i + 1) * T_CORE] = r["normt"].T
        logits[i * T_CORE:(i + 1) * T_CORE] = r["logits"]
    return (normed, logits)
